# revision 1
# baseline (speedup 1.0000x reference)
"""Trainium2 Bass kernel for nn_EnvironmentSpecificDecoder.

Data-parallel over batch B=32 across 8 NeuronCores (4 batches/core).
Per (b,t) slice (z [d=128, L=64]):
  stage1 : pair-packed fp32r matmuls: lhsT=[Zs_t0|Zs_t1] vs rhs=A gives
           z_aggT for two t's at once ([0:64]=t0, [64:128]=t1); the same
           with lhsT=[Zc_t0|Zc_t1] vs rhs=I transposes z_corrupt.
  S23    : fused signal projection + env MLP layer1 (W1s = W_sig @ W1[e],
           host-precomputed per env, dispatched on-device by regime via
           dynamic-offset DMA): h1T[h,(t,i)] = relu(W1s^T z_aggT + b1s).
  C1     : corrupt path h_cT = relu(Wc^T Zc^T + bc).
  S4+C2  : out2T[k,(t,i)] = W2[e]^T h1T  (+ Wo^T h_cT accumulated into the
           mu row of the same PSUM bank; bo+b2 folded into biases).
  post   : thin [2,512] evacuation per quad, SBUF->SBUF DMA compaction to
           dense [64, 512] tiles, softplus = ln(exp(x)+1), +0.01, then two
           strided output DMAs.

All matmuls run in float32r (E8M11): full fp32 range, 12-bit significand,
exact fp32 PSUM accumulation. Inputs are pre-rounded host-side.
"""
import os
import numpy as np

N_CORES = 8
NB = 4          # batches per core
T = 64
D = 128
L = 64
H = 256
H2 = 128
NE = 8

_CACHE = {}


def _round_fp32r(x: np.ndarray) -> np.ndarray:
    """Round fp32 array to E8M11 (float32r) with round-to-nearest-even."""
    u = np.ascontiguousarray(x, dtype=np.float32).view(np.uint32)
    keep = np.uint32(12)
    half = np.uint32(1 << 11)
    lsb = (u >> keep) & np.uint32(1)
    return ((u + (half - np.uint32(1) + lsb)) >> keep << keep).view(np.float32)


def _build():
    import concourse.bacc as bacc
    import concourse.bass as bass
    import concourse.mybir as mybir
    from concourse.tile import TileContext

    F32 = mybir.dt.float32
    F32R = mybir.dt.float32r
    AF = mybir.ActivationFunctionType
    ADD = mybir.AluOpType.add
    MAX = mybir.AluOpType.max

    nc = bacc.Bacc("TRN2", target_bir_lowering=False, debug=False)

    # zzi: [b, j, path(s/c), pair, t01*L]  (pair-packed lhsT slices)
    zzi_d = nc.dram_tensor("zzi", [NB, D, T * 2 * L], F32R, kind="ExternalInput")
    ai_d = nc.dram_tensor("ai", [D, 2 * D], F32R, kind="ExternalInput")
    reg_d = nc.dram_tensor("reg", [1, NB], mybir.dt.int32, kind="ExternalInput")
    w1s_d = nc.dram_tensor("w1s", [NE, D, H], F32R, kind="ExternalInput")
    b1s_d = nc.dram_tensor("b1s", [NE, D, 2], F32, kind="ExternalInput")
    w2p_d = nc.dram_tensor("w2p", [NE, D, 2, 2], F32R, kind="ExternalInput")
    b2a_d = nc.dram_tensor("b2a", [NE, 2, 1], F32, kind="ExternalInput")
    wc_d = nc.dram_tensor("wc", [D, H2], F32R, kind="ExternalInput")
    bc_d = nc.dram_tensor("bc", [H2, 1], F32, kind="ExternalInput")
    wo_d = nc.dram_tensor("wo", [H2, 1], F32R, kind="ExternalInput")

    mu_d = nc.dram_tensor("mu", [NB, T, D], F32, kind="ExternalOutput")
    sg_d = nc.dram_tensor("sg", [NB, T, D], F32, kind="ExternalOutput")

    with TileContext(nc) as tc:
        with (
            tc.tile_pool(name="const", bufs=1) as constp,
            tc.tile_pool(name="zz", bufs=2) as zzp,
            tc.tile_pool(name="ev", bufs=3) as evp,
            tc.tile_pool(name="stg", bufs=2) as stgp,
            tc.tile_pool(name="fin", bufs=1) as finp,
            tc.tile_pool(name="ps1", bufs=1, space="PSUM") as ps1,
            tc.tile_pool(name="ps23", bufs=1, space="PSUM") as ps23,
            tc.tile_pool(name="psc", bufs=1, space="PSUM") as psc,
            tc.tile_pool(name="ps4", bufs=1, space="PSUM") as ps4,
        ):
            # ---- static weights ----
            ai_sb = constp.tile([D, 2 * D], F32R)
            nc.sync.dma_start(ai_sb[:], ai_d[:])
            wc_sb = constp.tile([D, H2], F32R)       # Wc stacked twice
            nc.sync.dma_start(wc_sb[:], wc_d[:])
            wo_sb = constp.tile([H2, 1], F32R)
            nc.sync.dma_start(wo_sb[:], wo_d[:])
            bc_sb = constp.tile([H2, 1], F32)
            nc.sync.dma_start(bc_sb[:], bc_d[:])
            reg_sb = constp.tile([1, NB], mybir.dt.int32)
            nc.sync.dma_start(reg_sb[:], reg_d[:])

            # ---- per-batch dispatched weights (regime -> env) ----
            w1s_sb, b1s_sb, w2_sb, b2_sb = [], [], [], []
            for b in range(NB):
                e = nc.values_load(
                    reg_sb[0:1, b : b + 1],
                    engines=[mybir.EngineType.SP],
                    min_val=0, max_val=NE - 1,
                    skip_runtime_bounds_check=True,
                )
                w1 = constp.tile([D, H], F32R, name=f"w1s{b}", tag=f"w1s{b}")
                nc.sync.dma_start(
                    w1[:], w1s_d[bass.ds(e, 1)].rearrange("o p h -> (o p) h")
                )
                b1 = constp.tile([D, 2], F32, name=f"b1s{b}", tag=f"b1s{b}")
                nc.sync.dma_start(
                    b1[:], b1s_d[bass.ds(e, 1)].rearrange("o p h -> (o p) h")
                )
                w2 = constp.tile([D, 2, 2], F32R, name=f"w2{b}", tag=f"w2{b}")
                nc.sync.dma_start(
                    w2[:], w2p_d[bass.ds(e, 1)].rearrange("o p a k -> (o p) a k")
                )
                b2 = constp.tile([2, 1], F32, name=f"b2{b}", tag=f"b2{b}")
                nc.sync.dma_start(
                    b2[:], b2a_d[bass.ds(e, 1)].rearrange("o p k -> (o p) k")
                )
                w1s_sb.append(w1)
                b1s_sb.append(b1)
                w2_sb.append(w2)
                b2_sb.append(b2)

            st_mu = finp.tile([NB * 16, 512], F32)
            st_sig = finp.tile([NB * 16, 512], F32)

            for b in range(NB):
                zz = zzp.tile([D, T * 2 * L], F32R, tag="zz")
                nc.sync.dma_start(zz[:], zzi_d[b])

                for o in range(8):            # oct = 8 t's = 2 quads
                    # ---- stage 1: 4 signal pairs + 4 corrupt pairs ----
                    # p1 cols: kind*512 + qq*256 + tp*128 + i
                    p1 = ps1.tile([D, 1024], F32, tag="p1")
                    for qq in range(2):
                        for tp in range(2):
                            pr = o * 4 + qq * 2 + tp
                            nc.tensor.matmul(
                                p1[:, 256 * qq + 128 * tp :
                                   256 * qq + 128 * tp + 128],
                                zz[:, 128 * pr : 128 * (pr + 1)],
                                ai_sb[:, 0:128],
                                start=True, stop=True,
                            )
                            nc.tensor.matmul(
                                p1[:, 512 + 256 * qq + 128 * tp :
                                   512 + 256 * qq + 128 * tp + 128],
                                zz[:, 4096 + 128 * pr : 4096 + 128 * (pr + 1)],
                                ai_sb[:, 128:256],
                                start=True, stop=True,
                            )
                    # ---- stage-1 evacuation (rows 0:64 even-t, 64:128 odd) --
                    zzt = evp.tile([D, 1024], F32R, tag="zzt")
                    nc.vector.tensor_copy(zzt[:], p1[:])

                    # ---- S23: 4 matmuls N=512, one PSUM bank each ----
                    # p23 bank layout: (par*2+hh)*512
                    p23 = ps23.tile([D, 2048], F32, tag="p23")
                    for par in range(2):
                        for hh in range(2):
                            nc.tensor.matmul(
                                p23[:, (par * 2 + hh) * 512 :
                                    (par * 2 + hh) * 512 + 512],
                                w1s_sb[b][64 * par : 64 * par + 64,
                                          128 * hh : 128 * (hh + 1)],
                                zzt[64 * par : 64 * par + 64, 0:512],
                                start=True, stop=True,
                            )
                    # h1 cols: hh*1024 + qq*512 + par*256 + tp*128 + i
                    h1 = evp.tile([D, 2048], F32R, tag="h1")
                    p23v = p23[:].rearrange(
                        "p (par hh qq c) -> p par hh qq c", par=2, hh=2, qq=2)
                    h1v = h1[:].rearrange(
                        "p (hh qq par c) -> p hh qq par c", hh=2, qq=2, par=2)
                    for hh in range(2):
                        in_ap = p23v[:, :, hh].transpose([0, 2, 1, 3])
                        out_ap = h1v[:, hh]
                        if hh == 0:
                            nc.scalar.activation(
                                out_ap, in_ap, AF.Relu,
                                bias=b1s_sb[b][:, 0:1],
                            )
                        else:
                            nc.vector.tensor_scalar(
                                out_ap, in_ap,
                                b1s_sb[b][:, 1:2], 0.0, ADD, MAX,
                            )

                    # ---- C1: 2 matmuls N=512 (per-parity banks) ----
                    # hc cols: qq*512 + par*256 + tp*128 + i
                    hc = evp.tile([D, 1024], F32R, tag="hc")
                    hcv = hc[:].rearrange(
                        "p (qq par c) -> p qq par c", qq=2, par=2)
                    for par in range(2):
                        pc = psc.tile([D, 512], F32, tag="pc")
                        nc.tensor.matmul(
                            pc[:],
                            wc_sb[64 * par : 64 * par + 64, :],
                            zzt[64 * par : 64 * par + 64, 512:1024],
                            start=True, stop=True,
                        )
                        out_ap = hcv[:, :, par]
                        in_ap = pc[:].rearrange("p (qq c) -> p qq c", qq=2)
                        if (o + par) % 2 == 0:
                            nc.scalar.activation(
                                out_ap, in_ap, AF.Relu, bias=bc_sb[:, 0:1]
                            )
                        else:
                            nc.vector.tensor_scalar(
                                out_ap, in_ap, bc_sb[:, 0:1], 0.0, ADD, MAX
                            )

                    # ---- S4 + C2 per quad ----
                    st_raw = stgp.tile([2, 1024], F32, tag="st_raw")
                    for qq in range(2):
                        def t_rhs(ap512):
                            return ap512.rearrange(
                                "p (par tp i) -> p par tp i",
                                par=2, tp=2).transpose([0, 2, 1, 3])
                        p4 = ps4.tile([2, 512], F32, tag="p4")
                        nc.tensor.matmul(
                            p4[0:2, :], w2_sb[b][:, 0, :],
                            t_rhs(h1[:, 512 * qq : 512 * qq + 512]),
                            start=True, stop=False,
                        )
                        nc.tensor.matmul(
                            p4[0:1, :], wo_sb[:],
                            t_rhs(hc[:, 512 * qq : 512 * qq + 512]),
                            start=False, stop=False,
                        )
                        nc.tensor.matmul(
                            p4[0:2, :], w2_sb[b][:, 1, :],
                            t_rhs(h1[:, 1024 + 512 * qq : 1024 + 512 * qq + 512]),
                            start=False, stop=True,
                        )
                        nc.scalar.activation(
                            st_raw[:, 512 * qq : 512 * (qq + 1)], p4[:],
                            AF.Identity, bias=b2_sb[b][:, 0:1],
                        )
                    # ---- compaction: 2 quads -> dense rows ----
                    r0 = b * 16 + o * 2
                    for qq in range(2):
                        nc.sync.dma_start(
                            st_mu[r0 + qq : r0 + qq + 1, :],
                            st_raw[0:1, 512 * qq : 512 * (qq + 1)],
                        )
                        nc.sync.dma_start(
                            st_sig[r0 + qq : r0 + qq + 1, :],
                            st_raw[1:2, 512 * qq : 512 * (qq + 1)],
                        )

            # ---- sigma: softplus + 0.01 (dense) ----
            ex = finp.tile([NB * 16, 512], F32)
            nc.scalar.activation(ex[:], st_sig[:], AF.Exp)
            nc.scalar.activation(st_sig[:], ex[:], AF.Ln, bias=1.0)
            nc.vector.tensor_scalar_add(st_sig[:], st_sig[:], 0.01)

            # ---- outputs ----
            nc.sync.dma_start(
                mu_d[:].rearrange("b (qb tq) i -> (b qb) tq i", tq=4),
                st_mu[:].rearrange("p (tq i) -> p tq i", i=D),
            )
            nc.sync.dma_start(
                sg_d[:].rearrange("b (qb tq) i -> (b qb) tq i", tq=4),
                st_sig[:].rearrange("p (tq i) -> p tq i", i=D),
            )

    nc.compile()
    return nc


def _get_nc():
    if "nc" not in _CACHE:
        _CACHE["nc"] = _build()
    return _CACHE["nc"]


def _prepare_in_maps(z_signal, z_corrupt, A, regime, W_sig, b_sig, W1e, b1e,
                     W2e, b2e, Wc, bc, Wo, bo):
    z_signal = np.asarray(z_signal, dtype=np.float32)
    z_corrupt = np.asarray(z_corrupt, dtype=np.float32)
    A = np.asarray(A, dtype=np.float32)
    regime = np.asarray(regime)
    W_sig = np.asarray(W_sig, dtype=np.float32)
    b_sig = np.asarray(b_sig, dtype=np.float32)
    W1e = np.asarray(W1e, dtype=np.float32)
    b1e = np.asarray(b1e, dtype=np.float32)
    W2e = np.asarray(W2e, dtype=np.float32)
    b2e = np.asarray(b2e, dtype=np.float32)
    Wc = np.asarray(Wc, dtype=np.float32)
    bc = np.asarray(bc, dtype=np.float32)
    Wo = np.asarray(Wo, dtype=np.float32)
    bo = np.asarray(bo, dtype=np.float32)

    eidx = np.where(regime >= NE, 0, regime).astype(np.int32)

    # ---- host weight transforms (env tables, replicated to all cores) ----
    ai = _round_fp32r(np.concatenate([A, np.eye(D, dtype=np.float32)], axis=1))
    w1s_half = _round_fp32r(np.einsum("lh,ehk->elk", W_sig, W1e))  # [E, L, H]
    w1s = np.concatenate([w1s_half, w1s_half], axis=1)             # [E, D, H]
    b1s_full = np.einsum("h,ehk->ek", b_sig, W1e) + b1e            # [E, H]
    b1s = np.ascontiguousarray(
        b1s_full.reshape(NE, 2, D).transpose(0, 2, 1))             # [E, D, 2]
    w2p = _round_fp32r(
        np.ascontiguousarray(
            W2e.reshape(NE, 2, D, 2).transpose(0, 2, 1, 3)))       # [E, D, 2, 2]
    b2a = np.stack([b2e[:, 0] + bo[0], b2e[:, 1]], axis=1)[..., None]  # [E,2,1]
    wc_r = np.concatenate([_round_fp32r(Wc)] * 2, axis=0)          # [D, H2]
    wo_r = _round_fp32r(Wo)                                        # [H2, 1]
    bc_r = np.ascontiguousarray(bc[:, None])                       # [H2, 1]

    in_maps = []
    for c in range(N_CORES):
        b0 = c * NB
        zs = z_signal[b0 : b0 + NB]
        zc = z_corrupt[b0 : b0 + NB]
        # [nb, T, D, L] -> [nb, D, T/2, 2*L] pair-packed, stacked (sig, cor)
        def pack(z):
            zt = z.transpose(0, 2, 1, 3)                 # [nb, D, T, L]
            return zt.reshape(NB, D, T // 2, 2 * L)      # pairs
        zzi = np.concatenate([pack(zs), pack(zc)], axis=2)  # [nb, D, T, 2L]
        zzi = _round_fp32r(np.ascontiguousarray(
            zzi.reshape(NB, D, T * 2 * L)))
        in_maps.append({
            "zzi": zzi,
            "ai": ai,
            "reg": eidx[None, b0 : b0 + NB],
            "w1s": w1s,
            "b1s": b1s,
            "w2p": w2p,
            "b2a": b2a,
            "wc": wc_r,
            "bc": bc_r,
            "wo": wo_r,
        })
    return in_maps


def kernel(z_signal, z_corrupt, A, regime, W_sig, b_sig, W1e, b1e, W2e, b2e,
           Wc, bc, Wo, bo):
    from concourse.bass_utils import run_bass_kernel_spmd

    in_maps = _prepare_in_maps(z_signal, z_corrupt, A, regime, W_sig, b_sig,
                               W1e, b1e, W2e, b2e, Wc, bc, Wo, bo)
    nc = _get_nc()
    res = run_bass_kernel_spmd(nc, in_maps, core_ids=list(range(N_CORES)))

    mu = np.concatenate([r["mu"] for r in res.results], axis=0)
    sigma = np.concatenate([r["sg"] for r in res.results], axis=0)
    return mu, sigma


def run_traced(inputs_np):
    from concourse.bass_utils import run_bass_kernel_spmd

    in_maps = _prepare_in_maps(**inputs_np)
    nc = _get_nc()
    return run_bass_kernel_spmd(
        nc, in_maps, core_ids=list(range(N_CORES)), trace=True
    )



# revision 18
# speedup vs baseline: 1.4792x; 1.4792x over previous
"""Trainium2 Bass kernel for nn_EnvironmentSpecificDecoder (v2, bf16).

Data-parallel over batch B=32 across 8 NeuronCores (NB=4 batches/core).

Math (per b,t):  z_agg = A^T Z ;  h1 = relu(W1s^T z_agg + b1s) with
W1s = W_sig @ W1[env] host-fused;  out2 = W2[env]^T h1 (+ Wo^T relu(Wc^T
Zc + bc) into the mu row);  mu = out2[0]+b2' ; sigma = softplus(out2[1]+
b2[1]) + 0.01.

Device pipeline per oct (8 t's = 4 t-pairs), all matmuls bf16 (1 cyc/col,
FWL weight loads):
  stage1: 4 MMs, stationary = Z-pair [128j x 128(t01,l)], rhs = A (N=128)
          -> p1 = z_aggT [(t01,l), (pq,i)] in PSUM.  (z_corrupt needs no
          on-device transpose: host supplies ZcT packed.)
  S23   : 4 MMs [K=64(l), M=128, N=512] row-tiled 2x (t-parity tiles at
          partition 0/64 run concurrently) -> p23 = h1T pre-act.
  C1    : 2 MMs [K=64, M=128(h2), N=512] row-tiled 2x -> pc.
  evac  : ACT relu+bias p23 -> h1 (bf16); DVE relu+bias pc -> hc (bf16);
          DVE copy p1 -> zzt (bf16).
  S4+C2 : per t-parity, 3 accumulating MMs (W2 h-halves + Wo corrupt) at
          col-tile slot (0, 32s), s = (oct%2)*2+par: 4 slots of 2 octs
          share ONE psum bank at partition rows {0,32,64,96}+{0,1}.
  out   : one DVE bias-add evacuates all 4 slots [98,512] -> stb; mu rows
          DMA straight to HBM, sigma rows to st_sig; softplus (exp,
          ln(1+x), +0.01) once per core at the end.
"""
import numpy as np
import ml_dtypes

N_CORES = 8
NB = 4          # batches per core
T = 64
D = 128
L = 64
H = 256
H2 = 128
NE = 8

_CACHE = {}


def _bf16(x: np.ndarray) -> np.ndarray:
    return np.ascontiguousarray(x, dtype=np.float32).astype(ml_dtypes.bfloat16)


def _build():
    import concourse.bacc as bacc
    import concourse.bass as bass
    import concourse.mybir as mybir
    from concourse.tile import TileContext

    F32 = mybir.dt.float32
    BF16 = mybir.dt.bfloat16
    AF = mybir.ActivationFunctionType
    ADD = mybir.AluOpType.add
    MAX = mybir.AluOpType.max

    nc = bacc.Bacc("TRN2", target_bir_lowering=False, debug=False)

    zsp_d = nc.dram_tensor("zsp", [NB, D, T * L], BF16, kind="ExternalInput")
    zcp_d = nc.dram_tensor("zcp", [NB, D, T * L], BF16, kind="ExternalInput")
    ab_d = nc.dram_tensor("ab", [D, D], BF16, kind="ExternalInput")
    reg_d = nc.dram_tensor("reg", [1, NB], mybir.dt.int32, kind="ExternalInput")
    w1s_d = nc.dram_tensor("w1s", [NE, D, H], BF16, kind="ExternalInput")
    b1s_d = nc.dram_tensor("b1s", [NE, D, 2], F32, kind="ExternalInput")
    w2p_d = nc.dram_tensor("w2p", [NE, D, 2, 32], BF16, kind="ExternalInput")
    b2x_d = nc.dram_tensor("b2x", [NE, D, 1], F32, kind="ExternalInput")
    wc_d = nc.dram_tensor("wc", [D, H2], BF16, kind="ExternalInput")
    bc_d = nc.dram_tensor("bc", [H2, 1], F32, kind="ExternalInput")
    wo_d = nc.dram_tensor("wo", [H2, 32], BF16, kind="ExternalInput")

    mu_d = nc.dram_tensor("mu", [NB, T, D], F32, kind="ExternalOutput")
    sg_d = nc.dram_tensor("sg", [NB, T, D], F32, kind="ExternalOutput")

    with TileContext(nc) as tc:
        with (
            tc.tile_pool(name="const", bufs=1) as constp,
            tc.tile_pool(name="zs", bufs=2) as zsp_pool,
            tc.tile_pool(name="zc", bufs=2) as zcp_pool,
            tc.tile_pool(name="zzt", bufs=2) as zztp,
            tc.tile_pool(name="h1", bufs=2) as h1p,
            tc.tile_pool(name="hc", bufs=2) as hcp,
            tc.tile_pool(name="stb", bufs=2) as stp,
            tc.tile_pool(name="fin", bufs=1) as finp,
            tc.tile_pool(name="ps1", bufs=1, space="PSUM") as ps1,
            tc.tile_pool(name="ps23", bufs=1, space="PSUM") as ps23,
            tc.tile_pool(name="psc", bufs=1, space="PSUM") as psc,
            tc.tile_pool(name="ps4", bufs=1, space="PSUM") as ps4,
        ):
            # ---- static weights ----
            ab_sb = constp.tile([D, D], BF16)
            nc.sync.dma_start(ab_sb[:], ab_d[:])
            wc_sb = constp.tile([D, H2], BF16)       # Wc stacked twice (l rows)
            nc.sync.dma_start(wc_sb[:], wc_d[:])
            wo_sb = constp.tile([H2, 32], BF16)
            nc.sync.dma_start(wo_sb[:], wo_d[:])
            bc_sb = constp.tile([H2, 1], F32)
            nc.sync.dma_start(bc_sb[:], bc_d[:])
            reg_sb = constp.tile([1, NB], mybir.dt.int32)
            nc.sync.dma_start(reg_sb[:], reg_d[:])

            # ---- per-batch dispatched weights (regime -> env) ----
            w1s_sb, b1s_sb, w2_sb, b2x_sb = [], [], [], []
            for b in range(NB):
                e = nc.values_load(
                    reg_sb[0:1, b : b + 1],
                    engines=[mybir.EngineType.SP],
                    min_val=0, max_val=NE - 1,
                    skip_runtime_bounds_check=True,
                )
                w1 = constp.tile([D, H], BF16, name=f"w1s{b}", tag=f"w1s{b}")
                nc.sync.dma_start(
                    w1[:], w1s_d[bass.ds(e, 1)].rearrange("o p h -> (o p) h")
                )
                b1 = constp.tile([D, 2], F32, name=f"b1s{b}", tag=f"b1s{b}")
                nc.sync.dma_start(
                    b1[:], b1s_d[bass.ds(e, 1)].rearrange("o p h -> (o p) h")
                )
                w2 = constp.tile([D, 2, 32], BF16, name=f"w2{b}", tag=f"w2{b}")
                nc.sync.dma_start(
                    w2[:], w2p_d[bass.ds(e, 1)].rearrange("o p a k -> (o p) a k")
                )
                b2 = constp.tile([D, 1], F32, name=f"b2x{b}", tag=f"b2x{b}")
                nc.sync.dma_start(
                    b2[:], b2x_d[bass.ds(e, 1)].rearrange("o p k -> (o p) k")
                )
                w1s_sb.append(w1)
                b1s_sb.append(b1)
                w2_sb.append(w2)
                b2x_sb.append(b2)

            st_sig = finp.tile([NB * 16, 512], F32)

            for b in range(NB):
                zsb = zsp_pool.tile([D, T * L], BF16, tag="zsb")
                nc.sync.dma_start(zsb[:], zsp_d[b])
                zcb = zcp_pool.tile([D, T * L], BF16, tag="zcb")
                nc.sync.dma_start(zcb[:], zcp_d[b])

                for o in range(8):            # oct = 8 t's = 4 pairs
                    # ---- stage1: z_aggT = Zpair^T @ A, 4 pairs ----
                    p1 = ps1.tile([D, 512], F32, tag="p1")
                    for pq in range(4):
                        pr = o * 4 + pq
                        nc.tensor.matmul(
                            p1[:, 128 * pq : 128 * (pq + 1)],
                            zsb[:, 128 * pr : 128 * (pr + 1)],
                            ab_sb[:],
                            start=True, stop=True,
                        )
                    zzt = zztp.tile([D, 512], BF16, tag="zzt")
                    nc.vector.tensor_copy(zzt[:], p1[:])

                    # ---- S23: h1T pre-act, row-tiled over t-parity ----
                    # p23 cols: hh*1024 + par*512 + pq*128 + i
                    p23 = ps23.tile([D, 2048], F32, tag="p23")
                    for hh in range(2):
                        for par in range(2):
                            nc.tensor.matmul(
                                p23[:, hh * 1024 + par * 512 :
                                    hh * 1024 + par * 512 + 512],
                                w1s_sb[b][64 * par : 64 * par + 64,
                                          128 * hh : 128 * (hh + 1)],
                                zzt[64 * par : 64 * par + 64, :],
                                start=True, stop=True,
                            )

                    # ---- C1: h_cT pre-act, row-tiled over t-parity ----
                    pc = psc.tile([D, 1024], F32, tag="pc")
                    for par in range(2):
                        nc.tensor.matmul(
                            pc[:, par * 512 : par * 512 + 512],
                            wc_sb[64 * par : 64 * par + 64, :],
                            zcb[64 * par : 64 * par + 64,
                                o * 512 : o * 512 + 512],
                            start=True, stop=True,
                        )

                    # ---- evacuations (relu + bias, fp32 PSUM -> bf16) ----
                    h1 = h1p.tile([D, 2048], BF16, tag="h1")
                    nc.scalar.activation(
                        h1[:, 0:1024], p23[:, 0:1024], AF.Relu,
                        bias=b1s_sb[b][:, 0:1],
                    )
                    nc.scalar.activation(
                        h1[:, 1024:2048], p23[:, 1024:2048], AF.Relu,
                        bias=b1s_sb[b][:, 1:2],
                    )
                    hc = hcp.tile([D, 1024], BF16, tag="hc")
                    nc.vector.tensor_scalar(
                        hc[:], pc[:], bc_sb[:, 0:1], 0.0, ADD, MAX,
                    )

                    # ---- S4 + C2: col-tile slot per (oct%2, parity) ----
                    if o % 2 == 0:
                        p4 = ps4.tile([D, 512], F32, tag="p4")
                    for par in range(2):
                        s = (o % 2) * 2 + par
                        r = 32 * s
                        nc.tensor.matmul(
                            p4[r : r + 32, :], w2_sb[b][:, 0, :],
                            h1[:, par * 512 : par * 512 + 512],
                            start=True, stop=False,
                            tile_position=(0, r),
                        )
                        nc.tensor.matmul(
                            p4[r : r + 32, :], wo_sb[:],
                            hc[:, par * 512 : par * 512 + 512],
                            start=False, stop=False,
                            tile_position=(0, r),
                        )
                        nc.tensor.matmul(
                            p4[r : r + 32, :], w2_sb[b][:, 1, :],
                            h1[:, 1024 + par * 512 : 1024 + par * 512 + 512],
                            start=False, stop=True,
                            tile_position=(0, r),
                        )

                    if o % 2 == 1:
                        # one bias-add evacuates all 4 slots (rows between
                        # the 32-strips are never-written garbage)
                        stb = stp.tile([D, 512], F32, tag="stb")
                        nc.vector.tensor_scalar_add(
                            stb[:], p4[:], b2x_sb[b][:],
                        )
                        # mu rows {0,32,64,96} -> HBM directly
                        # t = oct*8 + pq*2 + par ; slot s = (oct%2, par)
                        muv = mu_d[b].rearrange(
                            "(o q p) i -> o p q i", q=4, p=2)
                        for s in range(4):
                            oct_, par = o - 1 + s // 2, s % 2
                            nc.sync.dma_start(
                                muv[oct_ : oct_ + 1, par],
                                stb[32 * s : 32 * s + 1, :].rearrange(
                                    "o (q i) -> o q i", i=D),
                            )
                            # sigma rows {1,33,65,97} -> softplus staging
                            nc.sync.dma_start(
                                st_sig[b * 16 + 2 * oct_ + par :
                                       b * 16 + 2 * oct_ + par + 1, :],
                                stb[32 * s + 1 : 32 * s + 2, :],
                            )

            # ---- sigma: softplus + 0.01 (dense) ----
            ex = finp.tile([NB * 16, 512], F32)
            nc.scalar.activation(ex[:], st_sig[:], AF.Exp)
            nc.scalar.activation(st_sig[:], ex[:], AF.Ln, bias=1.0)
            nc.vector.tensor_scalar_add(st_sig[:], st_sig[:], 0.01)
            sgv = sg_d[:].rearrange("b (o q p) i -> b o p q i", q=4, p=2)
            for b in range(NB):
                for o in range(8):
                    nc.sync.dma_start(
                        sgv[b, o],
                        st_sig[b * 16 + 2 * o : b * 16 + 2 * o + 2, :]
                        .rearrange("p (q i) -> p q i", i=D),
                    )

    nc.compile()
    return nc


def _get_nc():
    if "nc" not in _CACHE:
        _CACHE["nc"] = _build()
    return _CACHE["nc"]


def _prepare_in_maps(z_signal, z_corrupt, A, regime, W_sig, b_sig, W1e, b1e,
                     W2e, b2e, Wc, bc, Wo, bo):
    z_signal = np.asarray(z_signal, dtype=np.float32)
    z_corrupt = np.asarray(z_corrupt, dtype=np.float32)
    A = np.asarray(A, dtype=np.float32)
    regime = np.asarray(regime)
    W_sig = np.asarray(W_sig, dtype=np.float32)
    b_sig = np.asarray(b_sig, dtype=np.float32)
    W1e = np.asarray(W1e, dtype=np.float32)
    b1e = np.asarray(b1e, dtype=np.float32)
    W2e = np.asarray(W2e, dtype=np.float32)
    b2e = np.asarray(b2e, dtype=np.float32)
    Wc = np.asarray(Wc, dtype=np.float32)
    bc = np.asarray(bc, dtype=np.float32)
    Wo = np.asarray(Wo, dtype=np.float32)
    bo = np.asarray(bo, dtype=np.float32)

    eidx = np.where(regime >= NE, 0, regime).astype(np.int32)

    # ---- host weight transforms (env tables, replicated to all cores) ----
    ab = _bf16(A)
    w1s_half = np.einsum("lh,ehk->elk", W_sig, W1e)            # [E, L, H]
    w1s = _bf16(np.concatenate([w1s_half, w1s_half], axis=1))  # [E, D, H]
    b1s_full = np.einsum("h,ehk->ek", b_sig, W1e) + b1e        # [E, H]
    b1s = np.ascontiguousarray(
        b1s_full.reshape(NE, 2, D).transpose(0, 2, 1),
        dtype=np.float32)                                      # [E, D, 2]
    # S4 stationaries zero-padded M=2 -> M=32 so every partition of the
    # shared p4 bank gets written (no uninitialized PSUM, dense evac)
    w2p = np.zeros((NE, D, 2, 32), dtype=np.float32)
    w2p[:, :, :, 0:2] = W2e.reshape(NE, 2, D, 2).transpose(0, 2, 1, 3)
    w2p = _bf16(w2p)                                           # [E, D, 2, 32]
    b2x = np.zeros((NE, D, 1), dtype=np.float32)
    for s in range(4):
        b2x[:, 32 * s, 0] = b2e[:, 0] + bo[0]
        b2x[:, 32 * s + 1, 0] = b2e[:, 1]
    wc_r = _bf16(np.concatenate([Wc, Wc], axis=0))             # [D, H2]
    wo_r = np.zeros((H2, 32), dtype=np.float32)
    wo_r[:, 0] = Wo[:, 0]
    wo_r = _bf16(wo_r)                                         # [H2, 32]
    bc_r = np.ascontiguousarray(bc[:, None], dtype=np.float32)  # [H2, 1]

    in_maps = []
    for c in range(N_CORES):
        b0 = c * NB
        zs = z_signal[b0 : b0 + NB]       # [NB, T, D, L]
        zc = z_corrupt[b0 : b0 + NB]
        # signal: lhsT pairs [j, (pair, t01, l)]
        zsp = zs.reshape(NB, T // 2, 2, D, L).transpose(0, 3, 1, 2, 4)
        zsp = _bf16(zsp.reshape(NB, D, T * L))
        # corrupt: pre-transposed [(par, l), (pair, d)]
        zcp = zc.reshape(NB, T // 2, 2, D, L).transpose(0, 2, 4, 1, 3)
        zcp = _bf16(zcp.reshape(NB, D, T * L))
        in_maps.append({
            "zsp": zsp,
            "zcp": zcp,
            "ab": ab,
            "reg": eidx[None, b0 : b0 + NB],
            "w1s": w1s,
            "b1s": b1s,
            "w2p": w2p,
            "b2x": b2x,
            "wc": wc_r,
            "bc": bc_r,
            "wo": wo_r,
        })
    return in_maps


def kernel(z_signal, z_corrupt, A, regime, W_sig, b_sig, W1e, b1e, W2e, b2e,
           Wc, bc, Wo, bo):
    from concourse.bass_utils import run_bass_kernel_spmd

    in_maps = _prepare_in_maps(z_signal, z_corrupt, A, regime, W_sig, b_sig,
                               W1e, b1e, W2e, b2e, Wc, bc, Wo, bo)
    nc = _get_nc()
    res = run_bass_kernel_spmd(nc, in_maps, core_ids=list(range(N_CORES)))

    mu = np.concatenate([r["mu"] for r in res.results], axis=0)
    sigma = np.concatenate([r["sg"] for r in res.results], axis=0)
    return mu, sigma


def run_traced(inputs_np):
    from concourse.bass_utils import run_bass_kernel_spmd

    in_maps = _prepare_in_maps(**inputs_np)
    nc = _get_nc()
    return run_bass_kernel_spmd(
        nc, in_maps, core_ids=list(range(N_CORES)), trace=True
    )


# revision 20
# speedup vs baseline: 1.9816x; 1.3396x over previous
"""Trainium2 Bass kernel for nn_EnvironmentSpecificDecoder (v3, bf16 +
software-pipelined emission).

Data-parallel over batch B=32 across 8 NeuronCores (NB=4 batches/core).

Math (per b,t):  z_agg = A^T Z ;  h1 = relu(W1s^T z_agg + b1s) with
W1s = W_sig @ W1[env] host-fused;  out2 = W2[env]^T h1 (+ Wo^T relu(Wc^T
Zc + bc) into the mu row);  mu = out2[0]+b2' ; sigma = softplus(out2[1])
+ 0.01.

All matmuls bf16 (1 cyc/col + fast weight load).  Per oct (8 t's):
  stage1: 4 MMs, stationary = Z-pair, rhs = A (N=128) -> p1 = z_aggT.
          (z_corrupt is host-pretransposed; no on-device transposes.)
  S23   : 4 MMs [K=64, M=128, N=512] row-tiled 2x over t-parity into two
          2-bank PSUM chunks (hh0/hh1), each evacuated (relu+bias) as one
          [128,1024] op -- hh0 on ScalarE, hh1 on VectorE.
  C1    : 2 MMs row-tiled -> pc; relu+bias evac on ScalarE.
  S4+C2 : per t-parity 3 accumulating MMs (W2 h-halves + Wo) at col-tile
          slot (0,32s), s = (oct%2)*2+par; stationaries zero-padded
          M=2->32 so the 4 slots of 2 octs fill one whole PSUM bank.
  out   : one [128,512] VectorE bias-add -> stb; mu rows DMA straight to
          HBM; sigma rows to st_sig [32,1024] staged (q,p,i); softplus
          (exp, ln(1+x), +0.01) once at the end + single output DMA.

Emission is software-pipelined per slot o: PE runs stage1(o+1), S23(o),
C1(o), S4(o-1) so evacuations overlap matmuls of neighbouring octs.  All
8 z-input DMAs are issued up front (sync queue); compaction DMAs go to
the otherwise-idle GpSimd queue.
"""
import numpy as np
import ml_dtypes

N_CORES = 8
NB = 4          # batches per core
T = 64
D = 128
L = 64
H = 256
H2 = 128
NE = 8
NOCT = NB * 8   # global octs per core

_CACHE = {}


def _bf16(x: np.ndarray) -> np.ndarray:
    return np.ascontiguousarray(x, dtype=np.float32).astype(ml_dtypes.bfloat16)


def _build():
    import concourse.bacc as bacc
    import concourse.bass as bass
    import concourse.mybir as mybir
    from concourse.tile import TileContext

    F32 = mybir.dt.float32
    BF16 = mybir.dt.bfloat16
    AF = mybir.ActivationFunctionType
    ADD = mybir.AluOpType.add
    MAX = mybir.AluOpType.max

    nc = bacc.Bacc("TRN2", target_bir_lowering=False, debug=False)

    zsp_d = nc.dram_tensor("zsp", [NB, D, T * L], BF16, kind="ExternalInput")
    zcp_d = nc.dram_tensor("zcp", [NB, D, T * L], BF16, kind="ExternalInput")
    ab_d = nc.dram_tensor("ab", [D, D], BF16, kind="ExternalInput")
    reg_d = nc.dram_tensor("reg", [1, NB], mybir.dt.int32, kind="ExternalInput")
    w1s_d = nc.dram_tensor("w1s", [NE, D, H], BF16, kind="ExternalInput")
    b1s_d = nc.dram_tensor("b1s", [NE, D, 2], F32, kind="ExternalInput")
    w2p_d = nc.dram_tensor("w2p", [NE, D, 2, 32], BF16, kind="ExternalInput")
    b2x_d = nc.dram_tensor("b2x", [NE, D, 1], F32, kind="ExternalInput")
    wc_d = nc.dram_tensor("wc", [D, H2], BF16, kind="ExternalInput")
    bc_d = nc.dram_tensor("bc", [H2, 1], F32, kind="ExternalInput")
    wo_d = nc.dram_tensor("wo", [H2, 32], BF16, kind="ExternalInput")

    mu_d = nc.dram_tensor("mu", [NB, T, D], F32, kind="ExternalOutput")
    sg_d = nc.dram_tensor("sg", [NB, T, D], F32, kind="ExternalOutput")

    with TileContext(nc) as tc:
        with (
            tc.tile_pool(name="const", bufs=1) as constp,
            tc.tile_pool(name="zzt", bufs=2) as zztp,
            tc.tile_pool(name="h1", bufs=2) as h1p,
            tc.tile_pool(name="hc", bufs=2) as hcp,
            tc.tile_pool(name="stb", bufs=2) as stp,
            tc.tile_pool(name="fin", bufs=1) as finp,
            tc.tile_pool(name="ps1", bufs=1, space="PSUM") as ps1,
            tc.tile_pool(name="ps23", bufs=2, space="PSUM") as ps23,
            tc.tile_pool(name="psc", bufs=1, space="PSUM") as psc,
            tc.tile_pool(name="ps4", bufs=1, space="PSUM") as ps4,
        ):
            # ---- static weights + dispatch ----
            reg_sb = constp.tile([1, NB], mybir.dt.int32)
            nc.sync.dma_start(reg_sb[:], reg_d[:])
            ab_sb = constp.tile([D, D], BF16)
            nc.sync.dma_start(ab_sb[:], ab_d[:])
            wc_sb = constp.tile([D, H2], BF16)       # Wc stacked twice (l rows)
            nc.sync.dma_start(wc_sb[:], wc_d[:])
            wo_sb = constp.tile([H2, 32], BF16)
            nc.sync.dma_start(wo_sb[:], wo_d[:])
            bc_sb = constp.tile([H2, 1], F32)
            nc.sync.dma_start(bc_sb[:], bc_d[:])

            # first batch's activations, then weights, then the rest
            zsb, zcb = [None] * NB, [None] * NB
            zsb[0] = constp.tile([D, T * L], BF16, name="zsb0", tag="zsb0")
            nc.sync.dma_start(zsb[0][:], zsp_d[0])
            zcb[0] = constp.tile([D, T * L], BF16, name="zcb0", tag="zcb0")
            nc.sync.dma_start(zcb[0][:], zcp_d[0])

            w1s_sb, b1s_sb, w2_sb, b2x_sb = [], [], [], []
            for b in range(NB):
                e = nc.values_load(
                    reg_sb[0:1, b : b + 1],
                    engines=[mybir.EngineType.SP],
                    min_val=0, max_val=NE - 1,
                    skip_runtime_bounds_check=True,
                )
                w1 = constp.tile([D, H], BF16, name=f"w1s{b}", tag=f"w1s{b}")
                nc.sync.dma_start(
                    w1[:], w1s_d[bass.ds(e, 1)].rearrange("o p h -> (o p) h")
                )
                b1 = constp.tile([D, 2], F32, name=f"b1s{b}", tag=f"b1s{b}")
                nc.sync.dma_start(
                    b1[:], b1s_d[bass.ds(e, 1)].rearrange("o p h -> (o p) h")
                )
                w2 = constp.tile([D, 2, 32], BF16, name=f"w2{b}", tag=f"w2{b}")
                nc.sync.dma_start(
                    w2[:], w2p_d[bass.ds(e, 1)].rearrange("o p a k -> (o p) a k")
                )
                b2 = constp.tile([D, 1], F32, name=f"b2x{b}", tag=f"b2x{b}")
                nc.sync.dma_start(
                    b2[:], b2x_d[bass.ds(e, 1)].rearrange("o p k -> (o p) k")
                )
                w1s_sb.append(w1)
                b1s_sb.append(b1)
                w2_sb.append(w2)
                b2x_sb.append(b2)

            for b in range(1, NB):
                zsb[b] = constp.tile([D, T * L], BF16, name=f"zsb{b}",
                                     tag=f"zsb{b}")
                nc.sync.dma_start(zsb[b][:], zsp_d[b])
                zcb[b] = constp.tile([D, T * L], BF16, name=f"zcb{b}",
                                     tag=f"zcb{b}")
                nc.sync.dma_start(zcb[b][:], zcp_d[b])

            # sigma staging: row = (b, oct), col = (q, p, i) == (t%8, i)
            st_sig = finp.tile([NOCT, 1024], F32)

            zzt = [None] * NOCT    # per-oct handles for cross-slot refs
            h1 = [None] * NOCT
            hc = [None] * NOCT
            p4 = [None] * (NOCT // 2)
            muv = [
                mu_d[b].rearrange("(o q p) i -> o p q i", q=4, p=2)
                for b in range(NB)
            ]

            def stage1(o):
                b, oo = o // 8, o % 8
                p1 = ps1.tile([D, 512], F32, tag="p1")
                for pq in range(4):
                    pr = oo * 4 + pq
                    nc.tensor.matmul(
                        p1[:, 128 * pq : 128 * (pq + 1)],
                        zsb[b][:, 128 * pr : 128 * (pr + 1)],
                        ab_sb[:],
                        start=True, stop=True,
                    )
                zzt[o] = zztp.tile([D, 512], BF16, name=f"zzt{o}", tag="zzt")
                nc.vector.tensor_copy(zzt[o][:], p1[:])

            def s23(o):
                b = o // 8
                ck = [ps23.tile([D, 1024], F32, name=f"p23_{o}_{i}", tag="p23")
                      for i in range(2)]
                for hh in range(2):
                    for par in range(2):
                        nc.tensor.matmul(
                            ck[hh][:, par * 512 : par * 512 + 512],
                            w1s_sb[b][64 * par : 64 * par + 64,
                                      128 * hh : 128 * (hh + 1)],
                            zzt[o][64 * par : 64 * par + 64, :],
                            start=True, stop=True,
                        )
                # h1 cols: hh*1024 + par*512 + pq*128 + i
                h1[o] = h1p.tile([D, 2048], BF16, name=f"h1_{o}", tag="h1")
                nc.scalar.activation(
                    h1[o][:, 0:1024], ck[0][:], AF.Relu,
                    bias=b1s_sb[b][:, 0:1],
                )
                nc.vector.tensor_scalar(
                    h1[o][:, 1024:2048], ck[1][:],
                    b1s_sb[b][:, 1:2], 0.0, ADD, MAX,
                )

            def c1(o):
                b, oo = o // 8, o % 8
                pc = psc.tile([D, 1024], F32, name=f"pc{o}", tag="pc")
                for par in range(2):
                    nc.tensor.matmul(
                        pc[:, par * 512 : par * 512 + 512],
                        wc_sb[64 * par : 64 * par + 64, :],
                        zcb[b][64 * par : 64 * par + 64,
                               oo * 512 : oo * 512 + 512],
                        start=True, stop=True,
                    )
                hc[o] = hcp.tile([D, 1024], BF16, name=f"hc{o}", tag="hc")
                nc.scalar.activation(
                    hc[o][:], pc[:], AF.Relu, bias=bc_sb[:, 0:1],
                )

            def s4(o):
                b = o // 8
                if o % 2 == 0:
                    p4[o // 2] = ps4.tile([D, 512], F32, name=f"p4_{o}", tag="p4")
                pp = p4[o // 2]
                for par in range(2):
                    s = (o % 2) * 2 + par
                    r = 32 * s
                    nc.tensor.matmul(
                        pp[r : r + 32, :], w2_sb[b][:, 0, :],
                        h1[o][:, par * 512 : par * 512 + 512],
                        start=True, stop=False, tile_position=(0, r),
                    )
                    nc.tensor.matmul(
                        pp[r : r + 32, :], wo_sb[:],
                        hc[o][:, par * 512 : par * 512 + 512],
                        start=False, stop=False, tile_position=(0, r),
                    )
                    nc.tensor.matmul(
                        pp[r : r + 32, :], w2_sb[b][:, 1, :],
                        h1[o][:, 1024 + par * 512 : 1024 + par * 512 + 512],
                        start=False, stop=True, tile_position=(0, r),
                    )
                if o % 2 == 1:
                    stb = stp.tile([D, 512], F32, name=f"stb{o}", tag="stb")
                    nc.vector.tensor_scalar_add(stb[:], pp[:], b2x_sb[b][:])
                    for s in range(4):
                        oct_, par = (o - 1) % 8 + s // 2, s % 2
                        go = (o - 1) + s // 2        # global oct
                        nc.gpsimd.dma_start(
                            muv[b][oct_ : oct_ + 1, par],
                            stb[32 * s : 32 * s + 1, :].rearrange(
                                "o (q i) -> o q i", i=D),
                        )
                        nc.gpsimd.dma_start(
                            st_sig[go : go + 1, :].rearrange(
                                "r (q p i) -> r q p i", q=4, p=2
                            )[:, :, par],
                            stb[32 * s + 1 : 32 * s + 2, :].rearrange(
                                "o (q i) -> o q i", i=D),
                        )

            # ---- software-pipelined slots ----
            stage1(0)
            for o in range(NOCT + 1):
                if o + 1 < NOCT:
                    stage1(o + 1)
                if o < NOCT:
                    s23(o)
                    c1(o)
                if o >= 1:
                    s4(o - 1)

            # ---- sigma: softplus + 0.01 (dense) + single output DMA ----
            ex = finp.tile([NOCT, 1024], F32)
            nc.scalar.activation(ex[:], st_sig[:], AF.Exp)
            nc.scalar.activation(st_sig[:], ex[:], AF.Ln, bias=1.0)
            nc.vector.tensor_scalar_add(st_sig[:], st_sig[:], 0.01)
            nc.sync.dma_start(
                sg_d[:].rearrange("b (o s) i -> (b o) (s i)", s=8),
                st_sig[:],
            )

    nc.compile()
    return nc


def _get_nc():
    if "nc" not in _CACHE:
        _CACHE["nc"] = _build()
    return _CACHE["nc"]


def _prepare_in_maps(z_signal, z_corrupt, A, regime, W_sig, b_sig, W1e, b1e,
                     W2e, b2e, Wc, bc, Wo, bo):
    z_signal = np.asarray(z_signal, dtype=np.float32)
    z_corrupt = np.asarray(z_corrupt, dtype=np.float32)
    A = np.asarray(A, dtype=np.float32)
    regime = np.asarray(regime)
    W_sig = np.asarray(W_sig, dtype=np.float32)
    b_sig = np.asarray(b_sig, dtype=np.float32)
    W1e = np.asarray(W1e, dtype=np.float32)
    b1e = np.asarray(b1e, dtype=np.float32)
    W2e = np.asarray(W2e, dtype=np.float32)
    b2e = np.asarray(b2e, dtype=np.float32)
    Wc = np.asarray(Wc, dtype=np.float32)
    bc = np.asarray(bc, dtype=np.float32)
    Wo = np.asarray(Wo, dtype=np.float32)
    bo = np.asarray(bo, dtype=np.float32)

    eidx = np.where(regime >= NE, 0, regime).astype(np.int32)

    # ---- host weight transforms (env tables, replicated to all cores) ----
    ab = _bf16(A)
    w1s_half = np.einsum("lh,ehk->elk", W_sig, W1e)            # [E, L, H]
    w1s = _bf16(np.concatenate([w1s_half, w1s_half], axis=1))  # [E, D, H]
    b1s_full = np.einsum("h,ehk->ek", b_sig, W1e) + b1e        # [E, H]
    b1s = np.ascontiguousarray(
        b1s_full.reshape(NE, 2, D).transpose(0, 2, 1),
        dtype=np.float32)                                      # [E, D, 2]
    # S4 stationaries zero-padded M=2 -> M=32 so every partition of the
    # shared p4 bank gets written (no uninitialized PSUM, dense evac)
    w2p = np.zeros((NE, D, 2, 32), dtype=np.float32)
    w2p[:, :, :, 0:2] = W2e.reshape(NE, 2, D, 2).transpose(0, 2, 1, 3)
    w2p = _bf16(w2p)                                           # [E, D, 2, 32]
    b2x = np.zeros((NE, D, 1), dtype=np.float32)
    for s in range(4):
        b2x[:, 32 * s, 0] = b2e[:, 0] + bo[0]
        b2x[:, 32 * s + 1, 0] = b2e[:, 1]
    wc_r = _bf16(np.concatenate([Wc, Wc], axis=0))             # [D, H2]
    wo_r = np.zeros((H2, 32), dtype=np.float32)
    wo_r[:, 0] = Wo[:, 0]
    wo_r = _bf16(wo_r)                                         # [H2, 32]
    bc_r = np.ascontiguousarray(bc[:, None], dtype=np.float32)  # [H2, 1]

    in_maps = []
    for c in range(N_CORES):
        b0 = c * NB
        zs = z_signal[b0 : b0 + NB]       # [NB, T, D, L]
        zc = z_corrupt[b0 : b0 + NB]
        # signal: lhsT pairs [j, (pair, t01, l)]
        zsp = zs.reshape(NB, T // 2, 2, D, L).transpose(0, 3, 1, 2, 4)
        zsp = _bf16(zsp.reshape(NB, D, T * L))
        # corrupt: pre-transposed [(par, l), (pair, d)]
        zcp = zc.reshape(NB, T // 2, 2, D, L).transpose(0, 2, 4, 1, 3)
        zcp = _bf16(zcp.reshape(NB, D, T * L))
        in_maps.append({
            "zsp": zsp,
            "zcp": zcp,
            "ab": ab,
            "reg": eidx[None, b0 : b0 + NB],
            "w1s": w1s,
            "b1s": b1s,
            "w2p": w2p,
            "b2x": b2x,
            "wc": wc_r,
            "bc": bc_r,
            "wo": wo_r,
        })
    return in_maps


def kernel(z_signal, z_corrupt, A, regime, W_sig, b_sig, W1e, b1e, W2e, b2e,
           Wc, bc, Wo, bo):
    from concourse.bass_utils import run_bass_kernel_spmd

    in_maps = _prepare_in_maps(z_signal, z_corrupt, A, regime, W_sig, b_sig,
                               W1e, b1e, W2e, b2e, Wc, bc, Wo, bo)
    nc = _get_nc()
    res = run_bass_kernel_spmd(nc, in_maps, core_ids=list(range(N_CORES)))

    mu = np.concatenate([r["mu"] for r in res.results], axis=0)
    sigma = np.concatenate([r["sg"] for r in res.results], axis=0)
    return mu, sigma


def run_traced(inputs_np):
    from concourse.bass_utils import run_bass_kernel_spmd

    in_maps = _prepare_in_maps(**inputs_np)
    nc = _get_nc()
    return run_bass_kernel_spmd(
        nc, in_maps, core_ids=list(range(N_CORES)), trace=True
    )
